# revision 19
# baseline (speedup 1.0000x reference)
"""Trainium2 Bass kernel for a beam tree-ensemble (256 trees, depth 10,
complete binary trees, 256 features, 8 classes, batch 32768).

Data-parallel over batch across 8 NeuronCores. The full 10-level traversal
runs ON DEVICE:

  For each level d the candidate nodes of all trees form a table of
  M_d = 256 * 2^d (feature, threshold) pairs shared by every sample. The
  feature-value gather x[s, F_d[t, j]] uses the Pool/GPSIMD IndirectCopy
  instruction (the index list is shared across partitions, which matches
  its 16-partition-group shared-index semantics exactly), giving
  xg[s, (t,j)] for all candidates. Then bits = (xg >= TH_d) are reduced
  against a one-hot of the current node index (u8 select), and
  node = 2*node + bit. After 10 levels the u16 leaf-local index [0, 1024)
  is DMA'd out; the host expands leaves to class values (pure table
  lookup on data already resident host-side).

Layout: samples on partitions (32 tiles of 128), trees/candidates on the
free dimension, level tables chunked to 8192 candidates.
"""

import sys

sys.path.insert(0, "/opt/trn_rl_repo")

import numpy as np

import concourse.bass as bass
import concourse.tile as tile
from concourse import bacc, mybir, bass_utils
from concourse.alu_op_type import AluOpType
import bass_rust

NUM_TREES = 256
MAX_TREE_DEPTH = 10
NUM_NODES = 2 ** (MAX_TREE_DEPTH + 1) - 1  # 2047
N_INTERNAL = 2 ** MAX_TREE_DEPTH - 1       # 1023
N_FEATURES = 256
N_CLASSES = 8
BATCH = 32768
N_CORES = 8
BC = BATCH // N_CORES                      # 4096 samples per core
P = 128
NTILES = BC // P                           # 32 sample tiles per core
CHUNK = 4096                               # candidates per chunk
NK = 2                                     # sample tiles per DVE op group

F32 = mybir.dt.float32
U8 = mybir.dt.uint8
U16 = mybir.dt.uint16
I16 = mybir.dt.int16

_PROGRAM_CACHE = {}


def _split_multi_waits(nc):
    """This walrus build accepts at most one sem-wait per instruction; move
    extra waits onto single-wait NoOps placed before the owner."""
    ctr = 0
    for bb in nc.m.functions[0].blocks:
        new = []
        changed = False
        for inst in bb.instructions:
            si = inst.sync_info
            if si is not None and si.on_wait and len(si.on_wait) > 1:
                waits = list(si.on_wait)
                for w in waits[:-1]:
                    ctr += 1
                    n = mybir.InstNoOp(name=f"WSPLIT-{ctr}", ins=[], outs=[])
                    n.engine = inst.engine
                    n.sync_info = bass_rust.SyncInfo(on_wait=[w], on_update=[])
                    new.append(n)
                si.on_wait = [waits[-1]]
                changed = True
            new.append(inst)
        if changed:
            bb.instructions = new


def _chunk_schedule(T):
    """Yield (level, tree0, trees_per_chunk, chunk_elems) covering every
    internal tree level in table order."""
    sched = []
    for d in range(MAX_TREE_DEPTH):
        Md = T << d
        ch = min(CHUNK, Md)
        tpc = ch >> d
        for c in range(Md // ch):
            sched.append((d, c * tpc, tpc, ch))
    return sched


def build_program(T=NUM_TREES, ntiles=NTILES, split_waits=True):
    bc = ntiles * P
    total = T * N_INTERNAL
    wcols = total // 16

    nc = bacc.Bacc("TRN2", debug=False)
    x_d = nc.dram_tensor("xin", [bc, N_FEATURES], F32, kind="ExternalInput").ap()
    wf_d = nc.dram_tensor("wf", [16, wcols], I16, kind="ExternalInput").ap()
    th_d = nc.dram_tensor("th", [total], F32, kind="ExternalInput").ap()
    leaf_d = nc.dram_tensor("leaf", [bc, T], U16, kind="ExternalOutput").ap()

    with tile.TileContext(nc) as tc:
        with (
            tc.tile_pool(name="res", bufs=1) as res,
            tc.tile_pool(name="thp", bufs=2) as thp,
            tc.tile_pool(name="wfp", bufs=2) as wfp,
            tc.tile_pool(name="xgp", bufs=2) as xgp,
            tc.tile_pool(name="ohp", bufs=2) as ohp,
            tc.tile_pool(name="bbp", bufs=2) as bbp,
        ):
            x_all = res.tile([P, ntiles, N_FEATURES], F32)
            node = res.tile([P, ntiles, T], U16)
            bit = res.tile([P, ntiles, T], U8)
            iota16 = res.tile([P, 512], U16)

            nc.sync.dma_start(x_all[:], x_d.rearrange("(k p) f -> p k f", p=P))
            nc.gpsimd.memset(node[:], 0)
            nc.gpsimd.iota(
                iota16[:], pattern=[[1, 512]], base=0, channel_multiplier=0,
                allow_small_or_imprecise_dtypes=True,
            )

            woff = 0
            toff = 0
            for d, t0, tpc, ch in _chunk_schedule(T):
                j = 1 << d
                wc = ch // 16
                # wrapped gather indices for this chunk, replicated to the
                # 8 GPSIMD 16-partition groups
                wfb = wfp.tile([P, CHUNK // 16], I16, name=f"wf_{d}_{t0}", tag="wf")
                for g in range(8):
                    nc.sync.dma_start(
                        wfb[16 * g : 16 * (g + 1), :wc], wf_d[:, woff : woff + wc]
                    )
                # thresholds broadcast to all partitions
                thb = thp.tile([P, CHUNK], F32, name=f"th_{d}_{t0}", tag="th")
                nc.sync.dma_start(
                    thb[:, :ch], th_d[toff : toff + ch].partition_broadcast(P)
                )
                for k0 in range(0, ntiles, NK):
                    nkk = min(NK, ntiles - k0)
                    xg = xgp.tile([P, NK, CHUNK], F32, name=f"xg_{d}_{t0}_{k0}", tag="xg")
                    oh = ohp.tile([P, NK, CHUNK], U8, name=f"oh_{d}_{t0}_{k0}", tag="oh")
                    bb = bbp.tile([P, NK, CHUNK], U8, name=f"bb_{d}_{t0}_{k0}", tag="bb")
                    for kk in range(nkk):
                        nc.gpsimd.ap_gather(
                            xg[:, kk, :ch], x_all[:, k0 + kk, :], wfb[:, :wc],
                            channels=P, num_elems=N_FEATURES, d=1, num_idxs=ch,
                        )
                    # bb = (xg >= th)
                    nc.vector.tensor_tensor(
                        bb[:, :nkk, :ch],
                        xg[:, :nkk, :ch],
                        thb[:, :ch].unsqueeze(1).broadcast_to([P, nkk, ch]),
                        AluOpType.is_ge,
                    )
                    # oh = onehot(node == j)
                    nc.vector.tensor_tensor(
                        oh[:, :nkk, :ch].rearrange("p k (t j) -> p k t j", j=j),
                        node[:, k0 : k0 + nkk, t0 : t0 + tpc]
                        .unsqueeze(3)
                        .broadcast_to([P, nkk, tpc, j]),
                        iota16[:, :j]
                        .unsqueeze(1)
                        .unsqueeze(1)
                        .broadcast_to([P, nkk, tpc, j]),
                        AluOpType.is_equal,
                    )
                    # oh *= bb ; bit = max_j oh
                    nc.vector.tensor_tensor(
                        oh[:, :nkk, :ch],
                        oh[:, :nkk, :ch],
                        bb[:, :nkk, :ch],
                        AluOpType.mult,
                    )
                    nc.vector.tensor_reduce(
                        bit[:, k0 : k0 + nkk, t0 : t0 + tpc],
                        oh[:, :nkk, :ch].rearrange("p k (t j) -> p k t j", j=j),
                        axis=mybir.AxisListType.X,
                        op=AluOpType.max,
                    )
                woff += wc
                toff += ch
                if t0 + tpc == T:  # last chunk of this level
                    nc.vector.tensor_scalar(
                        node[:], node[:], 2, 0, AluOpType.mult, AluOpType.add
                    )
                    nc.vector.tensor_tensor(node[:], node[:], bit[:], AluOpType.add)

            nc.sync.dma_start(leaf_d.rearrange("(k p) t -> p k t", p=P), node[:])

    nc.compile()
    if split_waits:  # needed for HW; the sim race detector rejects the NoOps
        _split_multi_waits(nc)
    return nc


_RUNNER_CACHE = {}

# program inputs that are identical on every core (tables); uploaded once
# and replicated device-side instead of 8x over the tunnel
REPLICATED_INPUTS = frozenset({"wf", "th"})


def run_device(nc, full_inputs):
    """Execute the 8-core SPMD program via PJRT like
    bass2jax.run_bass_via_pjrt, with two changes: the jitted executable is
    cached across calls, and the donated output buffers are created on
    device with jnp.zeros instead of being uploaded from the host (the
    kernel writes every output element, so the zero-fill is never
    observable).

    ``full_inputs`` maps tensor name -> global array whose axis 0
    concatenates the per-core shards.
    """
    import jax
    import jax.numpy as jnp
    from jax.sharding import Mesh, PartitionSpec, NamedSharding
    from jax.experimental.shard_map import shard_map
    from concourse import bass2jax as B

    key = id(nc)
    if key not in _RUNNER_CACHE:
        B.install_neuronx_cc_hook()
        partition_name = (
            nc.partition_id_tensor.name if nc.partition_id_tensor else None
        )
        in_names = []
        out_names = []
        out_avals = []
        out_shapes = []
        for alloc in nc.m.functions[0].allocations:
            if not isinstance(alloc, mybir.MemoryLocationSet):
                continue
            name = alloc.memorylocations[0].name
            if alloc.kind == "ExternalInput":
                if name != partition_name:
                    in_names.append(name)
            elif alloc.kind == "ExternalOutput":
                shape = tuple(alloc.tensor_shape)
                dtype = mybir.dt.np(alloc.dtype)
                out_avals.append(jax.core.ShapedArray(shape, dtype))
                out_names.append(name)
                out_shapes.append((shape, dtype))
        n_params = len(in_names)
        n_outs = len(out_names)
        all_names = list(in_names) + list(out_names)
        if partition_name is not None:
            all_names.append(partition_name)

        def _body(*args):
            operands = list(args)
            if partition_name is not None:
                operands.append(B.partition_id_tensor())
            outs = B._bass_exec_p.bind(
                *operands,
                out_avals=tuple(out_avals),
                in_names=tuple(all_names),
                out_names=tuple(out_names),
                lowering_input_output_aliases=(),
                sim_require_finite=True,
                sim_require_nnan=True,
                nc=nc,
            )
            return tuple(outs)

        devices = jax.devices()[:N_CORES]
        mesh = Mesh(np.asarray(devices), ("core",))
        # inputs whose name is in REPLICATED_INPUTS are passed whole to
        # every core (one tunnel upload); the rest shard over axis 0
        specs = tuple(
            PartitionSpec() if n in REPLICATED_INPUTS else PartitionSpec("core")
            for n in in_names
        ) + (PartitionSpec("core"),) * n_outs
        out_specs = (PartitionSpec("core"),) * n_outs
        donate = tuple(range(n_params, n_params + n_outs))
        sharded = jax.jit(
            shard_map(
                _body, mesh=mesh, in_specs=specs, out_specs=out_specs,
                check_rep=False,
            ),
            donate_argnums=donate,
            keep_unused=True,
        )
        sharding = NamedSharding(mesh, PartitionSpec("core"))
        state = {
            "in_names": in_names,
            "out_names": out_names,
            "sharded": sharded,
            "out_shapes": out_shapes,
            "sharding": sharding,
            # output buffers recycled as the next call's donated scratch;
            # every output element is written by the kernel, so contents
            # are irrelevant. Zeros are uploaded only on the first call.
            "bufs": None,
        }
        _RUNNER_CACHE[key] = state

    state = _RUNNER_CACHE[key]
    import jax as _jax

    if state["bufs"] is None:
        state["bufs"] = [
            _jax.device_put(
                np.zeros((N_CORES * shape[0],) + shape[1:], dtype),
                state["sharding"],
            )
            for shape, dtype in state["out_shapes"]
        ]
    out_arrs = state["sharded"](
        *[full_inputs[n] for n in state["in_names"]], *state["bufs"]
    )
    state["bufs"] = list(out_arrs)
    in_names, out_names = state["in_names"], state["out_names"]
    return {
        name: np.asarray(out_arrs[i]) for i, name in enumerate(out_names)
    }


def host_tables(features, thresholds, T=NUM_TREES):
    """Per-level (tree-major) threshold table and wrapped feature-index
    blocks matching IndirectCopy's 16-partition interleave."""
    feats = features.reshape(T, NUM_NODES)
    thr = thresholds.reshape(T, NUM_NODES)
    wf_parts = []
    th_parts = []
    for d, t0, tpc, ch in _chunk_schedule(T):
        lo = (1 << d) - 1
        hi = (2 << d) - 1
        Fd = feats[t0 : t0 + tpc, lo:hi].reshape(-1).astype(np.int16)
        Td = thr[t0 : t0 + tpc, lo:hi].reshape(-1).astype(np.float32)
        wf_parts.append(Fd.reshape(ch // 16, 16).T)
        th_parts.append(Td)
    wf = np.ascontiguousarray(np.concatenate(wf_parts, axis=1))
    th = np.concatenate(th_parts)
    return wf, th


def kernel(x, lefts, rights, features, thresholds, values, nodes_offset):
    x = np.asarray(x, dtype=np.float32)
    features = np.asarray(features, dtype=np.int32)
    thresholds = np.asarray(thresholds, dtype=np.float32)
    values = np.asarray(values, dtype=np.float32)

    wf, th = host_tables(features, thresholds)

    if "prog" not in _PROGRAM_CACHE:
        _PROGRAM_CACHE["prog"] = build_program()
    nc = _PROGRAM_CACHE["prog"]

    full_inputs = {"xin": np.ascontiguousarray(x), "wf": wf, "th": th}
    res = None
    last_err = None
    for _attempt in range(3):
        try:
            res = run_device(nc, full_inputs)
            break
        except Exception as e:  # transient NRT device-unrecoverable after crashes
            last_err = e
    if res is None:
        raise last_err

    leaf = res["leaf"].astype(np.int64)  # [B, T] leaf-local in [0, 1024)

    vleaf = np.ascontiguousarray(
        values.reshape(NUM_TREES, NUM_NODES, N_CLASSES)[:, N_INTERNAL:, :]
    )
    tix = np.arange(NUM_TREES)[None, :]
    return vleaf[tix, leaf]  # [B, T, 8] float32
